# revision 24
# baseline (speedup 1.0000x reference)
"""Trainium2 Bass kernel for nn_Downsample: stride-2 3x3 conv with ternary weights + bias.

Full inputs in, full output out. Internally: data-parallel over batch across 8
NeuronCores (4 images/core), weights replicated.

Math: out[b,co,ho,wo] = sum_{ci,kh,kw} x[b,ci,2ho-1+kh,2wo-1+kw] * wq[co,ci,kh,kw] + bias[co]
with wq = ternary(clip(weight)) in {-1,0,+1}.

Device formulation: for each output tile [128 co x 512 pixels], accumulate
27 matmuls (9 taps x 3 ci-blocks, K=128 each) in one PSUM bank. The stride-2
spatial gather is expressed directly in the matmul moving-operand access
pattern over a (65,65) zero-padded fp16 image in SBUF; no on-chip gather or
cast needed. x is pre-cast to fp16 on host (ternary weights are exact in
fp16; measured absmax-relative error vs f32 reference: 2.1e-4).

Performance (per core, 648 matmuls of K=128/M=128/N=512 fp16):
  - pure matmul stream floor: 138.2 us (N cycles @ 2.4 GHz)
  - TimelineSim cost model:   148.4 us
  - measured on HW (differential over a repeat loop): ~149-175 us
    depending on terminal load; clean-period minimum ~149 us.
Startup is hidden by critical-path-first DMA ordering (first 288KB weight
slice + quarter image), PE warm-up matmuls burn the HAM cold-clock window
during the initial DMA wait, and the first/last output tiles use
quarter-height PSUM groups to shorten the dependency ramp and drain tail.
"""

import os
import sys
from contextlib import ExitStack

import numpy as np

sys.path.insert(0, "/opt/trn_rl_repo")

import concourse.mybir as mybir  # noqa: E402
import concourse.tile as tile  # noqa: E402
from concourse import bacc, bass_utils  # noqa: E402

# This container's axon build has no NTFF-profile hook module; stub it so a
# trace=True / BASS_TRACE=1 run degrades to no-trace instead of crashing.
try:
    import antenv.axon_hooks  # noqa: F401
except ImportError:
    import types as _types

    _stub = _types.ModuleType("antenv.axon_hooks")
    _stub.get_axon_ntff_profile_hook = lambda: None
    sys.modules["antenv.axon_hooks"] = _stub

N_CORES = 8
B, C, H, W = 32, 384, 64, 64
HO, WO = 32, 32
BPC = B // N_CORES  # images per core
CB = C // 128  # channel blocks (3)
HP, WP = H + 1, W + 1  # zero-padded (left/top only; right/bottom never read)
NTAPS = 9

_cached = {}


def _build_nc(reps=1, quarter_first=True, w_tap_split=True, x_chunks3=True,
              warmup_mms=24, quarter_last=True, planes=False):
    nc = bacc.Bacc("TRN2", target_bir_lowering=False, debug=False, num_devices=N_CORES)
    if planes:
        # space-to-depth: x split into 4 parity planes of the padded image so
        # every matmul moving-operand AP has a contiguous inner dimension
        x_ap = nc.dram_tensor("x", [BPC, C, 2, 2, 33, 33], mybir.dt.float16, kind="ExternalInput").ap()
    else:
        x_ap = nc.dram_tensor("x", [BPC, C, HP, WP], mybir.dt.float16, kind="ExternalInput").ap()
    # w layout: [ci, ob*1152 + tap*128 + co_in_block] so the first output-channel
    # block's weights arrive with a small 288KB DMA (critical path to first matmul)
    w_ap = nc.dram_tensor("w", [C, CB * NTAPS * 128], mybir.dt.float16, kind="ExternalInput").ap()
    b_ap = nc.dram_tensor("bias", [CB, 128], mybir.dt.float32, kind="ExternalInput").ap()
    o_ap = nc.dram_tensor("out", [BPC, C, HO, WO], mybir.dt.float32, kind="ExternalOutput").ap()

    with tile.TileContext(nc) as tc, ExitStack() as ctx:
        wpool = ctx.enter_context(tc.tile_pool(name="wpool", bufs=CB * CB))
        xpool = ctx.enter_context(tc.tile_pool(name="xpool", bufs=BPC * CB))
        opool = ctx.enter_context(tc.tile_pool(name="opool", bufs=6))
        bpool = ctx.enter_context(tc.tile_pool(name="bpool", bufs=1))
        psum = ctx.enter_context(tc.tile_pool(name="psum", bufs=8, space="PSUM"))

        def body():
            # PE warm-up: the HAM clock gate holds the PE at 1.2 GHz until it
            # has been busy ~3.4us. Burn that window on zero matmuls while the
            # first DMAs are still in flight, so real matmuls start at 2.4 GHz.
            if warmup_mms:
                wu = bpool.tile([128, 512], mybir.dt.float16, name="wu", tag="wu")
                nc.vector.memset(wu[:, :], 0)
                wu_ps = psum.tile([128, 512], mybir.dt.float32, name="wu_ps", tag="ps")
                for i in range(warmup_mms):
                    nc.tensor.matmul(wu_ps[:, :128], wu[:, :128], wu[:, :128],
                                     start=True, stop=True)

            # --- DMA issue order = critical path first ---
            # First matmuls need: w(cb=0,ob=0) taps 0-2, then x(b=0) rows 0..16.
            # Weight DMAs lead (small); x image-0 arrives in three row chunks.
            x_sb, w_sb = {}, {}

            def load_x(b, cb, h0, h1):
                if planes:
                    if (b, cb) not in x_sb:
                        x_sb[(b, cb)] = xpool.tile(
                            [128, 2, 2, 33, 33], mybir.dt.float16, name=f"x_{b}_{cb}", tag="x"
                        )
                    xt = x_sb[(b, cb)]
                    # h0:h1 is a padded-image row range; map to plane rows
                    # covering it: plane row a holds padded rows 2a/2a+1
                    a0, a1 = h0 // 2, min((h1 + 1) // 2, 33)
                    nc.sync.dma_start(
                        xt[:, :, :, a0:a1, :],
                        x_ap[b, cb * 128 : (cb + 1) * 128, :, :, a0:a1, :],
                    )
                    return
                if (b, cb) not in x_sb:
                    x_sb[(b, cb)] = xpool.tile(
                        [128, HP, WP], mybir.dt.float16, name=f"x_{b}_{cb}", tag="x"
                    )
                xt = x_sb[(b, cb)]
                nc.sync.dma_start(
                    xt[:, h0:h1, :], x_ap[b, cb * 128 : (cb + 1) * 128, h0:h1, :]
                )

            def load_w(cb, ob, t0=0, t1=NTAPS):
                if (cb, ob) not in w_sb:
                    w_sb[(cb, ob)] = wpool.tile(
                        [128, NTAPS * 128], mybir.dt.float16, name=f"w_{cb}_{ob}", tag="w"
                    )
                wt = w_sb[(cb, ob)]
                nc.sync.dma_start(
                    wt[:, t0 * 128 : t1 * 128],
                    w_ap[cb * 128 : (cb + 1) * 128, (ob * NTAPS + t0) * 128 : (ob * NTAPS + t1) * 128],
                )

            if w_tap_split:
                load_w(0, 0, 0, 3)
            else:
                load_w(0, 0)
            if x_chunks3:
                load_x(0, 0, 0, 17)
                if w_tap_split:
                    load_w(0, 0, 3, NTAPS)
                load_x(0, 1, 0, 17)
                load_x(0, 2, 0, 17)
                for cb in range(CB):
                    load_x(0, cb, 17, 33)
                load_w(1, 0)
                load_w(2, 0)
                for cb in range(CB):
                    load_x(0, cb, 33, HP)
            else:
                load_x(0, 0, 0, 33)
                if w_tap_split:
                    load_w(0, 0, 3, NTAPS)
                load_x(0, 1, 0, 33)
                load_w(1, 0)
                load_x(0, 2, 0, 33)
                load_w(2, 0)
                for cb in range(CB):
                    load_x(0, cb, 33, HP)
            for ob in range(1, CB):
                for cb in range(CB):
                    load_w(cb, ob)
            bias_sb = bpool.tile([128, CB], mybir.dt.float32, name="bias_sb", tag="bias")
            for ob in range(CB):
                nc.sync.dma_start(bias_sb[:, ob : ob + 1], b_ap[ob, :].unsqueeze(1))
            for b in range(1, BPC):
                for cb in range(CB):
                    for h0, h1 in ((0, 33), (33, HP)):
                        load_x(b, cb, h0, h1)

            def group(b, ob, ho0, nh):
                # one PSUM accumulation group covering output rows [ho0, ho0+nh)
                pt = psum.tile([128, nh, WO], mybir.dt.float32, name=f"ps_{b}_{ob}_{ho0}",
                               tag="ps", padded_shape=[128, 16, WO])
                mm = 0
                for cb in range(CB):
                    xt = x_sb[(b, cb)]
                    for kh in range(3):
                        for kw in range(3):
                            if planes:
                                ph, a0 = kh % 2, kh // 2
                                pw, b0 = kw % 2, kw // 2
                                rhs = xt[:, ph, pw, a0 + ho0 : a0 + ho0 + nh, b0 : b0 + 32]
                            else:
                                rhs = xt[:, 2 * ho0 + kh : 2 * ho0 + kh + 2 * nh - 1 : 2, kw : kw + 63 : 2]
                            lhsT = w_sb[(cb, ob)][:, (kh * 3 + kw) * 128 : (kh * 3 + kw) * 128 + 128]
                            nc.tensor.matmul(
                                pt[:, :, :], lhsT, rhs,
                                start=(mm == 0), stop=(mm == NTAPS * CB - 1),
                            )
                            mm += 1
                ot = opool.tile([128, nh, WO], mybir.dt.float32, name=f"o_{b}_{ob}_{ho0}",
                                tag="o", padded_shape=[128, 16, WO])
                nc.vector.tensor_scalar_add(ot[:, :, :], pt[:, :, :], bias_sb[:, ob : ob + 1])
                nc.sync.dma_start(o_ap[b, ob * 128 : (ob + 1) * 128, ho0 : ho0 + nh, :], ot[:, :, :])

            for b in range(BPC):
                for ob in range(CB):
                    first = b == 0 and ob == 0
                    last = b == BPC - 1 and ob == CB - 1
                    if (quarter_first and first) or (quarter_last and last):
                        # quarter-height groups: at the start compute begins once
                        # the first x row-chunk lands; at the end the drain tail
                        # (DVE + store) of the final group is halved
                        for ho0 in (0, 8, 16, 24):
                            group(b, ob, ho0, 8)
                    else:
                        for ho0 in (0, 16):
                            group(b, ob, ho0, 16)

        if reps == 1:
            body()
        else:
            with tc.For_i(0, reps, 1):
                body()

    nc.compile()
    return nc


def _prep_inputs(x, weight, bias, planes=False):
    wq = np.clip(np.asarray(weight, dtype=np.float32), -1.0, 1.0)
    wq = np.where(wq > 0.001, 1.0, np.where(wq < -0.001, -1.0, 0.0)).astype(np.float16)
    # wT[ci, ob*9*128 + (kh*3+kw)*128 + cq] = wq[ob*128+cq, ci, kh, kw]
    wT = np.ascontiguousarray(
        wq.reshape(CB, 128, C, 3, 3).transpose(2, 0, 3, 4, 1).reshape(C, CB * NTAPS * 128)
    )

    xp = np.zeros((B, C, HP, WP), dtype=np.float16)
    xp[:, :, 1:, 1:] = np.asarray(x)

    if planes:
        # plane[ph,pw][a,b] = xp[2a+ph, 2b+pw]
        xpl = np.zeros((B, C, 2, 2, 33, 33), dtype=np.float16)
        for ph in range(2):
            for pw in range(2):
                src = xp[:, :, ph::2, pw::2]
                xpl[:, :, ph, pw, : src.shape[2], : src.shape[3]] = src
        xp = xpl

    b32 = np.ascontiguousarray(np.asarray(bias, dtype=np.float32).reshape(CB, 128))
    return xp, wT, b32


PLANES = False  # space-to-depth x layout (contiguous-inner matmul APs)


def _run(x, weight, bias, trace=False):
    if "nc" not in _cached:
        _cached["nc"] = _build_nc(planes=PLANES)
    nc = _cached["nc"]

    xp, wT, b32 = _prep_inputs(x, weight, bias, planes=PLANES)
    in_maps = [
        {"x": np.ascontiguousarray(xp[c * BPC : (c + 1) * BPC]), "w": wT, "bias": b32}
        for c in range(N_CORES)
    ]
    res = bass_utils.run_bass_kernel_spmd(
        nc, in_maps, core_ids=list(range(N_CORES)), trace=trace,
    )
    out = np.concatenate([res.results[c]["out"] for c in range(N_CORES)], axis=0)
    return out, res


def kernel(x, time_emb=None, y=None, weight=None, bias=None, **_):
    out, _res = _run(x, weight, bias, trace=bool(int(os.environ.get("KERNEL_TRACE", "0"))))
    return out
